# revision 12
# baseline (speedup 1.0000x reference)
"""V3: output-sharded fp8 gather with shared-uniq dedup per 256-out supergroup.

out[n, o] = sum_k x[n, idx[o, k]] * w[o, k] + b[o]

Per core (512 outputs): 2 super-groups (sg) of 256 outputs = out-tiles A, B.
The sg's unique features are gathered once, laid out in 21 tiles of 128:
  A-region  slots [0, 1024):    A-only + dup'd excess-shared (A weights)
  M-region  slots [1024, 1664): shared (<= 640, excess duplicated out)
  B-region  slots [1664, 2688): dup'd excess-shared (B weights) + B-only
so each out-tile contracts a CONTIGUOUS 13-tile window:
  A: tiles 0..12,  B: tiles 8..20     [verified for the seeded input]
Expansion uses host-built sparse lhsT tiles [128 uniq, 128 outs]:
  psum[128 outs, 512 rows] = sum_u lhsT_u.T @ g[window_u, rows]
Gather volume: 2688 fp8 lines/sg-half; PE 26 tile-matmuls per sg vs 32
undeduped. The timing loop unrolls UNROLL bodies per For_i iteration so the
iteration-boundary barrier + gather refill amortizes across bodies.
"""

import sys

import numpy as np
import ml_dtypes

for _p in ("/opt/trn_rl_repo", "/opt/pypackages"):
    if _p not in sys.path:
        sys.path.append(_p)

N = 4096
IN_F = 4096
OUT_F = 4096
K = 16
NCORES = 8
OSLAB = OUT_F // NCORES       # 512 outputs per core
SG = 256                      # outputs per dedup super-group
NSG = OSLAB // SG             # 2 super-groups per core
UTILES = 21                   # uniq-feature tiles of 128 per super-group
UPAD = UTILES * 128           # 2688 padded uniq slots
AREG = 1024                   # A-region slots [0, 1024)
MREG = 640                    # shared region slots [1024, 1664)
WIN = ((0, 13), (8, 13))      # (start tile, num tiles) for out-tiles A, B
LTILES = sum(n for _, n in WIN)  # 26 lhsT tiles per sg
H = 1                         # row halves (full 4096-B rows per descriptor)
HROWS = N // H                # 4096 rows per half
# gather call tap ranges: EXACTLY one call per SWDGE queue per stage (fewer
# queues starves stream concurrency, >4 calls reuses a ring intra-stage; both
# measured slower). Near-equal sizes so the 4 streams finish together.
CALLS = [(0, 768), (768, 640), (1408, 640), (2048, 640)]
UNROLL = 8                    # bodies per For_i iteration

_CACHE = {}


def _build(reps: int = 1):
    import concourse.bacc as bacc
    import concourse.mybir as mybir
    import concourse.tile as tile

    dt = mybir.dt
    nc = bacc.Bacc("TRN2", debug=False, num_devices=NCORES,
                   enable_partition_id=False, num_swdge_queues=4)

    xt = nc.dram_tensor("xt", [IN_F, N], dt.float8e3, kind="ExternalInput")
    idxs = nc.dram_tensor("idxs", [128, NSG * (UPAD // 16)], dt.int16,
                          kind="ExternalInput")
    # lhsT laid out host-side exactly as the SBUF tile:
    # [p, ((s*LTILES) + t)*128 + o]; t 0..12 = A window, 13..25 = B window
    lhsT_d = nc.dram_tensor("lhsT", [128, NSG * LTILES * 128], dt.bfloat16,
                            kind="ExternalInput")
    bias = nc.dram_tensor("bias", [128, NSG * 2], dt.float32,
                          kind="ExternalInput")
    outT = nc.dram_tensor("outT", [OSLAB, N], dt.float16, kind="ExternalOutput")
    nc.dram_tensor("repstag", [1, reps], dt.float32, kind="ExternalOutput")

    with tile.TileContext(nc) as tc:
        with (
            tc.tile_pool(name="singles", bufs=1) as singles,
            tc.tile_pool(name="gpool", bufs=2) as gpool,
            tc.tile_pool(name="ppool", bufs=8, space="PSUM") as ppool,
            tc.tile_pool(name="opool", bufs=2) as opool,
        ):
            idxs_sb = singles.tile([128, NSG * (UPAD // 16)], dt.int16)
            nc.sync.dma_start(idxs_sb[:], idxs[:])
            lhsT_sb = singles.tile([128, NSG, LTILES, 128], dt.bfloat16)
            nc.sync.dma_start(lhsT_sb[:], lhsT_d[:])
            bias_sb = singles.tile([128, NSG * 2], dt.float32)
            nc.sync.dma_start(bias_sb[:], bias[:])

            def body(bi=0):
                for s in range(NSG):
                    for h in range(H):
                        stage = bi * NSG + s
                        g = gpool.tile([128, UTILES, HROWS], dt.float8e3)
                        for k, (cstart, clen) in enumerate(CALLS):
                            nc.gpsimd.dma_gather(
                                g[:, cstart // 128:(cstart + clen) // 128, :],
                                xt[:, h * HROWS:(h + 1) * HROWS],
                                idxs_sb[:, s * (UPAD // 16) + cstart // 16:
                                        s * (UPAD // 16) + (cstart + clen) // 16],
                                clen, clen, HROWS,
                                elem_step=N,
                                queue_num=(k + stage) % 4,
                            )
                        lbase = 0
                        for ot, (ustart, ucnt) in enumerate(WIN):
                            o = opool.tile([128, HROWS], dt.float16)
                            for ch in range(HROWS // 512):
                                p = ppool.tile([128, 512], dt.float32)
                                for u in range(ucnt):
                                    nc.tensor.matmul(
                                        p[:], lhsT_sb[:, s, lbase + u, :],
                                        g[:, ustart + u,
                                          ch * 512:(ch + 1) * 512],
                                        start=(u == 0), stop=(u == ucnt - 1))
                                nc.scalar.activation(
                                    o[:, ch * 512:(ch + 1) * 512], p[:],
                                    mybir.ActivationFunctionType.Identity,
                                    bias=bias_sb[:, s * 2 + ot:s * 2 + ot + 1])
                            nc.sync.dma_start(
                                outT[(s * 2 + ot) * 128:(s * 2 + ot + 1) * 128,
                                     h * HROWS:(h + 1) * HROWS],
                                o[:])
                            lbase += ucnt

            nloop, rem = divmod(reps, UNROLL)
            if nloop > 0:
                with tc.For_i(0, nloop, 1):
                    for bi in range(UNROLL):
                        body(bi)
            for bi in range(rem):
                body(bi)

    nc.compile()
    return nc


def _prep_inputs(x, in_index_per_out, weight, bias):
    """Host prep: fp8 xT (replicated) + per-core dedup tables."""
    idx = np.asarray(in_index_per_out).astype(np.int64)
    w = np.asarray(weight).astype(np.float32)
    b = np.asarray(bias).astype(np.float32)

    xT = np.ascontiguousarray(np.asarray(x).astype(np.float32).T
                              .astype(ml_dtypes.float8_e3m4))  # (IN_F, N)

    idxs_l, lhsT_l, bias_l = [], [], []
    for d in range(NCORES):
        idx_wraps = []
        lh_core = np.zeros((128, NSG, LTILES, 128), dtype=np.float32)
        for s in range(NSG):
            lo = d * OSLAB + s * SG
            iA, iB = idx[lo:lo + 128], idx[lo + 128:lo + 256]
            wA, wB = w[lo:lo + 128], w[lo + 128:lo + 256]
            uA, uB = np.unique(iA.ravel()), np.unique(iB.ravel())
            shared = np.intersect1d(uA, uB)
            Aonly = np.setdiff1d(uA, shared)
            Bonly = np.setdiff1d(uB, shared)
            ndup = max(0, len(shared) - MREG)
            excess, shmid = shared[:ndup], shared[ndup:]
            nAo, nBo, nSh = len(Aonly), len(Bonly), len(shmid)
            assert nAo + ndup <= AREG, (d, s, nAo, ndup)
            assert nBo + ndup <= UPAD - AREG - MREG, (d, s, nBo, ndup)
            # slot layout: [Aonly dupA pad | shmid pad | dupB Bonly pad]
            taps = np.zeros(UPAD, dtype=np.int64)
            taps[:nAo] = Aonly
            taps[nAo:nAo + ndup] = excess
            taps[AREG:AREG + nSh] = shmid
            taps[AREG + MREG:AREG + MREG + ndup] = excess
            taps[AREG + MREG + ndup:AREG + MREG + ndup + nBo] = Bonly
            # per-out-tile position maps (slot index for each feature)
            posA = np.full(IN_F, -1, dtype=np.int64)
            posA[Aonly] = np.arange(nAo)
            posA[excess] = nAo + np.arange(ndup)
            posA[shmid] = AREG + np.arange(nSh)
            posB = np.full(IN_F, -1, dtype=np.int64)
            posB[shmid] = AREG + np.arange(nSh)
            posB[excess] = AREG + MREG + np.arange(ndup)
            posB[Bonly] = AREG + MREG + ndup + np.arange(nBo)
            for ot, (ustart, ucnt), pm, io, wo in (
                    (0, WIN[0], posA, iA, wA), (1, WIN[1], posB, iB, wB)):
                Am = np.zeros((ucnt * 128, 128), dtype=np.float32)
                pos = pm[io] - ustart * 128       # (128, 16)
                assert (pos >= 0).all() and (pos < ucnt * 128).all(), (d, s, ot)
                for ol in range(128):
                    for m in range(K):
                        Am[pos[ol, m], ol] += wo[ol, m]
                lbase = 0 if ot == 0 else WIN[0][1]
                lh_core[:, s, lbase:lbase + ucnt, :] = \
                    Am.reshape(ucnt, 128, 128).transpose(1, 0, 2)
            idx_wraps.append(np.tile(
                taps.astype(np.int16).reshape(-1, 16).T, (8, 1)))
        idxs_l.append(np.ascontiguousarray(
            np.concatenate(idx_wraps, axis=1)).astype(np.int16))
        lhsT_l.append(np.ascontiguousarray(
            lh_core.reshape(128, NSG * LTILES * 128))
            .astype(ml_dtypes.bfloat16))
        bias_l.append(np.ascontiguousarray(
            b[d * OSLAB:(d + 1) * OSLAB].reshape(NSG * 2, 128).T))
    return xT, idxs_l, lhsT_l, bias_l


def _in_maps(inputs):
    xT, idxs_l, lhsT_l, bias_l = _prep_inputs(
        inputs["x"], inputs["in_index_per_out"], inputs["weight"],
        inputs["bias"])
    return [
        {"xt": xT, "idxs": idxs_l[d], "lhsT": lhsT_l[d], "bias": bias_l[d]}
        for d in range(NCORES)
    ]


def kernel(x, in_index_per_out, weight, bias):
    from concourse import bass_utils

    in_maps = _in_maps({"x": x, "in_index_per_out": in_index_per_out,
                        "weight": weight, "bias": bias})

    if "nc" not in _CACHE:
        _CACHE["nc"] = _build(reps=1)
    nc = _CACHE["nc"]
    res = bass_utils.run_bass_kernel_spmd(nc, in_maps,
                                          core_ids=list(range(NCORES)))
    out = np.empty((N, OUT_F), dtype=np.float32)
    for d in range(NCORES):
        out[:, d * OSLAB:(d + 1) * OSLAB] = \
            res.results[d]["outT"].astype(np.float32).T
    return out


# revision 15
# speedup vs baseline: 1.0116x; 1.0116x over previous
"""Output-sharded fp8 gather with shared-uniq dedup per 256-out supergroup.

out[n, o] = sum_k x[n, idx[o, k]] * w[o, k] + b[o]

Per core (512 outputs): 2 super-groups (sg) of 256 outputs = out-tiles A, B.
The sg's unique features are gathered once, laid out in 21 tiles of 128:
  A-region  slots [0, 1024):    A-only + dup'd excess-shared (A weights)
  M-region  slots [1024, 1664): shared (<= 640, excess duplicated out)
  B-region  slots [1664, 2688): dup'd excess-shared (B weights) + B-only
so each out-tile contracts a CONTIGUOUS 13-tile window:
  A: tiles 0..12,  B: tiles 8..20     [verified for the seeded input]
Expansion uses host-built sparse lhsT tiles [128 uniq, 128 outs]:
  psum[128 outs, 512 rows] = sum_u lhsT_u.T @ g[window_u, rows]
Gather volume: 2688 fp8 lines/sg-half; PE 26 tile-matmuls per sg vs 32
undeduped. The timing loop unrolls UNROLL bodies per For_i iteration so the
iteration-boundary barrier + gather refill amortizes across bodies.
"""

import sys

import numpy as np
import ml_dtypes

for _p in ("/opt/trn_rl_repo", "/opt/pypackages"):
    if _p not in sys.path:
        sys.path.append(_p)

N = 4096
IN_F = 4096
OUT_F = 4096
K = 16
NCORES = 8
OSLAB = OUT_F // NCORES       # 512 outputs per core
SG = 256                      # outputs per dedup super-group
NSG = OSLAB // SG             # 2 super-groups per core
UTILES = 21                   # uniq-feature tiles of 128 per super-group
UPAD = UTILES * 128           # 2688 padded uniq slots
AREG = 1024                   # A-region slots [0, 1024)
MREG = 640                    # shared region slots [1024, 1664)
WIN = ((0, 13), (8, 13))      # (start tile, num tiles) for out-tiles A, B
LTILES = sum(n for _, n in WIN)  # 26 lhsT tiles per sg
H = 1                         # row halves (full 4096-B rows per descriptor)
HROWS = N // H                # 4096 rows per half
# gather call tap ranges: EXACTLY one call per SWDGE queue per stage (fewer
# queues starves stream concurrency, >4 calls reuses a ring intra-stage; both
# measured slower). Near-equal sizes so the 4 streams finish together.
CALLS = [(0, 640), (640, 640), (1280, 640), (1920, 768)]
UNROLL = 8                    # bodies per For_i iteration

_CACHE = {}


def _build(reps: int = 1):
    import concourse.bacc as bacc
    import concourse.mybir as mybir
    import concourse.tile as tile

    dt = mybir.dt
    nc = bacc.Bacc("TRN2", debug=False, num_devices=NCORES,
                   enable_partition_id=False, num_swdge_queues=4)

    xt = nc.dram_tensor("xt", [IN_F, N], dt.float8e3, kind="ExternalInput")
    idxs = nc.dram_tensor("idxs", [128, NSG * (UPAD // 16)], dt.int16,
                          kind="ExternalInput")
    # lhsT laid out host-side exactly as the SBUF tile:
    # [p, ((s*LTILES) + t)*128 + o]; t 0..12 = A window, 13..25 = B window
    lhsT_d = nc.dram_tensor("lhsT", [128, NSG * LTILES * 128], dt.bfloat16,
                            kind="ExternalInput")
    bias = nc.dram_tensor("bias", [128, NSG * 2], dt.float32,
                          kind="ExternalInput")
    outT = nc.dram_tensor("outT", [OSLAB, N], dt.float16, kind="ExternalOutput")
    nc.dram_tensor("repstag", [1, reps], dt.float32, kind="ExternalOutput")

    with tile.TileContext(nc) as tc:
        with (
            tc.tile_pool(name="singles", bufs=1) as singles,
            tc.tile_pool(name="gpool", bufs=2) as gpool,
            tc.tile_pool(name="ppool", bufs=8, space="PSUM") as ppool,
            tc.tile_pool(name="opool", bufs=2) as opool,
        ):
            idxs_sb = singles.tile([128, NSG * (UPAD // 16)], dt.int16)
            nc.sync.dma_start(idxs_sb[:], idxs[:])
            lhsT_sb = singles.tile([128, NSG, LTILES, 128], dt.bfloat16)
            nc.sync.dma_start(lhsT_sb[:], lhsT_d[:])
            bias_sb = singles.tile([128, NSG * 2], dt.float32)
            nc.sync.dma_start(bias_sb[:], bias[:])

            def body(bi=0):
                for s in range(NSG):
                    for h in range(H):
                        g = gpool.tile([128, UTILES, HROWS], dt.float8e3)
                        for k, (cstart, clen) in enumerate(CALLS):
                            nc.gpsimd.dma_gather(
                                g[:, cstart // 128:(cstart + clen) // 128, :],
                                xt[:, h * HROWS:(h + 1) * HROWS],
                                idxs_sb[:, s * (UPAD // 16) + cstart // 16:
                                        s * (UPAD // 16) + (cstart + clen) // 16],
                                clen, clen, HROWS,
                                elem_step=N,
                                queue_num=k,
                            )
                        lbase = 0
                        for ot, (ustart, ucnt) in enumerate(WIN):
                            o = opool.tile([128, HROWS], dt.float16)
                            for ch in range(HROWS // 512):
                                p = ppool.tile([128, 512], dt.float32)
                                for u in range(ucnt):
                                    nc.tensor.matmul(
                                        p[:], lhsT_sb[:, s, lbase + u, :],
                                        g[:, ustart + u,
                                          ch * 512:(ch + 1) * 512],
                                        start=(u == 0), stop=(u == ucnt - 1))
                                nc.scalar.activation(
                                    o[:, ch * 512:(ch + 1) * 512], p[:],
                                    mybir.ActivationFunctionType.Identity,
                                    bias=bias_sb[:, s * 2 + ot:s * 2 + ot + 1])
                            nc.sync.dma_start(
                                outT[(s * 2 + ot) * 128:(s * 2 + ot + 1) * 128,
                                     h * HROWS:(h + 1) * HROWS],
                                o[:])
                            lbase += ucnt

            nloop, rem = divmod(reps, UNROLL)
            if nloop > 0:
                with tc.For_i(0, nloop, 1):
                    for bi in range(UNROLL):
                        body(bi)
            for bi in range(rem):
                body(bi)

    nc.compile()
    return nc


def _prep_inputs(x, in_index_per_out, weight, bias):
    """Host prep: fp8 xT (replicated) + per-core dedup tables."""
    idx = np.asarray(in_index_per_out).astype(np.int64)
    w = np.asarray(weight).astype(np.float32)
    b = np.asarray(bias).astype(np.float32)

    xT = np.ascontiguousarray(np.asarray(x).astype(np.float32).T
                              .astype(ml_dtypes.float8_e3m4))  # (IN_F, N)

    idxs_l, lhsT_l, bias_l = [], [], []
    for d in range(NCORES):
        idx_wraps = []
        lh_core = np.zeros((128, NSG, LTILES, 128), dtype=np.float32)
        for s in range(NSG):
            lo = d * OSLAB + s * SG
            iA, iB = idx[lo:lo + 128], idx[lo + 128:lo + 256]
            wA, wB = w[lo:lo + 128], w[lo + 128:lo + 256]
            uA, uB = np.unique(iA.ravel()), np.unique(iB.ravel())
            shared = np.intersect1d(uA, uB)
            Aonly = np.setdiff1d(uA, shared)
            Bonly = np.setdiff1d(uB, shared)
            ndup = max(0, len(shared) - MREG)
            excess, shmid = shared[:ndup], shared[ndup:]
            nAo, nBo, nSh = len(Aonly), len(Bonly), len(shmid)
            assert nAo + ndup <= AREG, (d, s, nAo, ndup)
            assert nBo + ndup <= UPAD - AREG - MREG, (d, s, nBo, ndup)
            # slot layout: [Aonly dupA pad | shmid pad | dupB Bonly pad]
            taps = np.zeros(UPAD, dtype=np.int64)
            taps[:nAo] = Aonly
            taps[nAo:nAo + ndup] = excess
            taps[AREG:AREG + nSh] = shmid
            taps[AREG + MREG:AREG + MREG + ndup] = excess
            taps[AREG + MREG + ndup:AREG + MREG + ndup + nBo] = Bonly
            # per-out-tile position maps (slot index for each feature)
            posA = np.full(IN_F, -1, dtype=np.int64)
            posA[Aonly] = np.arange(nAo)
            posA[excess] = nAo + np.arange(ndup)
            posA[shmid] = AREG + np.arange(nSh)
            posB = np.full(IN_F, -1, dtype=np.int64)
            posB[shmid] = AREG + np.arange(nSh)
            posB[excess] = AREG + MREG + np.arange(ndup)
            posB[Bonly] = AREG + MREG + ndup + np.arange(nBo)
            for ot, (ustart, ucnt), pm, io, wo in (
                    (0, WIN[0], posA, iA, wA), (1, WIN[1], posB, iB, wB)):
                Am = np.zeros((ucnt * 128, 128), dtype=np.float32)
                pos = pm[io] - ustart * 128       # (128, 16)
                assert (pos >= 0).all() and (pos < ucnt * 128).all(), (d, s, ot)
                for ol in range(128):
                    for m in range(K):
                        Am[pos[ol, m], ol] += wo[ol, m]
                lbase = 0 if ot == 0 else WIN[0][1]
                lh_core[:, s, lbase:lbase + ucnt, :] = \
                    Am.reshape(ucnt, 128, 128).transpose(1, 0, 2)
            idx_wraps.append(np.tile(
                taps.astype(np.int16).reshape(-1, 16).T, (8, 1)))
        idxs_l.append(np.ascontiguousarray(
            np.concatenate(idx_wraps, axis=1)).astype(np.int16))
        lhsT_l.append(np.ascontiguousarray(
            lh_core.reshape(128, NSG * LTILES * 128))
            .astype(ml_dtypes.bfloat16))
        bias_l.append(np.ascontiguousarray(
            b[d * OSLAB:(d + 1) * OSLAB].reshape(NSG * 2, 128).T))
    return xT, idxs_l, lhsT_l, bias_l


def _in_maps(inputs):
    xT, idxs_l, lhsT_l, bias_l = _prep_inputs(
        inputs["x"], inputs["in_index_per_out"], inputs["weight"],
        inputs["bias"])
    return [
        {"xt": xT, "idxs": idxs_l[d], "lhsT": lhsT_l[d], "bias": bias_l[d]}
        for d in range(NCORES)
    ]


def kernel(x, in_index_per_out, weight, bias):
    from concourse import bass_utils

    in_maps = _in_maps({"x": x, "in_index_per_out": in_index_per_out,
                        "weight": weight, "bias": bias})

    if "nc" not in _CACHE:
        _CACHE["nc"] = _build(reps=1)
    nc = _CACHE["nc"]
    res = bass_utils.run_bass_kernel_spmd(nc, in_maps,
                                          core_ids=list(range(NCORES)))
    out = np.empty((N, OUT_F), dtype=np.float32)
    for d in range(NCORES):
        out[:, d * OSLAB:(d + 1) * OSLAB] = \
            res.results[d]["outT"].astype(np.float32).T
    return out


# revision 16
# speedup vs baseline: 1.0299x; 1.0181x over previous
"""Output-sharded fp8 gather with shared-uniq dedup per 256-out supergroup.

out[n, o] = sum_k x[n, idx[o, k]] * w[o, k] + b[o]

Per core (512 outputs): 2 super-groups (sg) of 256 outputs = out-tiles A, B.
The sg's unique features are gathered once, laid out in 21 tiles of 128:
  A-region  slots [0, 1024):    A-only + dup'd excess-shared (A weights)
  M-region  slots [1024, 1664): shared (<= 640, excess duplicated out)
  B-region  slots [1664, 2688): dup'd excess-shared (B weights) + B-only
so each out-tile contracts a CONTIGUOUS 13-tile window:
  A: tiles 0..12,  B: tiles 8..20     [verified for the seeded input]
Expansion uses host-built sparse lhsT tiles [128 uniq, 128 outs]:
  psum[128 outs, 512 rows] = sum_u lhsT_u.T @ g[window_u, rows]
Gather volume: 2688 fp8 lines/sg-half; PE 26 tile-matmuls per sg vs 32
undeduped. The timing loop unrolls UNROLL bodies per For_i iteration so the
iteration-boundary barrier + gather refill amortizes across bodies.
"""

import sys

import numpy as np
import ml_dtypes

for _p in ("/opt/trn_rl_repo", "/opt/pypackages"):
    if _p not in sys.path:
        sys.path.append(_p)

N = 4096
IN_F = 4096
OUT_F = 4096
K = 16
NCORES = 8
OSLAB = OUT_F // NCORES       # 512 outputs per core
SG = 256                      # outputs per dedup super-group
NSG = OSLAB // SG             # 2 super-groups per core
UTILES = 21                   # uniq-feature tiles of 128 per super-group
UPAD = UTILES * 128           # 2688 padded uniq slots
AREG = 1024                   # A-region slots [0, 1024)
MREG = 640                    # shared region slots [1024, 1664)
WIN = ((0, 13), (8, 13))      # (start tile, num tiles) for out-tiles A, B
LTILES = sum(n for _, n in WIN)  # 26 lhsT tiles per sg
H = 1                         # row halves (full 4096-B rows per descriptor)
HROWS = N // H                # 4096 rows per half
# gather call tap ranges: EXACTLY one call per SWDGE queue per stage (fewer
# queues starves stream concurrency, >4 calls reuses a ring intra-stage; both
# measured slower). Near-equal sizes so the 4 streams finish together.
CALLS = [(0, 640), (640, 640), (1280, 640), (1920, 768)]
UNROLL = 10                   # bodies per For_i iteration

_CACHE = {}


def _build(reps: int = 1):
    import concourse.bacc as bacc
    import concourse.mybir as mybir
    import concourse.tile as tile

    dt = mybir.dt
    nc = bacc.Bacc("TRN2", debug=False, num_devices=NCORES,
                   enable_partition_id=False, num_swdge_queues=4)

    xt = nc.dram_tensor("xt", [IN_F, N], dt.float8e3, kind="ExternalInput")
    idxs = nc.dram_tensor("idxs", [128, NSG * (UPAD // 16)], dt.int16,
                          kind="ExternalInput")
    # lhsT laid out host-side exactly as the SBUF tile:
    # [p, ((s*LTILES) + t)*128 + o]; t 0..12 = A window, 13..25 = B window
    lhsT_d = nc.dram_tensor("lhsT", [128, NSG * LTILES * 128], dt.bfloat16,
                            kind="ExternalInput")
    bias = nc.dram_tensor("bias", [128, NSG * 2], dt.float32,
                          kind="ExternalInput")
    outT = nc.dram_tensor("outT", [OSLAB, N], dt.float16, kind="ExternalOutput")
    nc.dram_tensor("repstag", [1, reps], dt.float32, kind="ExternalOutput")

    with tile.TileContext(nc) as tc:
        with (
            tc.tile_pool(name="singles", bufs=1) as singles,
            tc.tile_pool(name="gpool", bufs=2) as gpool,
            tc.tile_pool(name="ppool", bufs=8, space="PSUM") as ppool,
            tc.tile_pool(name="opool", bufs=2) as opool,
        ):
            idxs_sb = singles.tile([128, NSG * (UPAD // 16)], dt.int16)
            nc.sync.dma_start(idxs_sb[:], idxs[:])
            lhsT_sb = singles.tile([128, NSG, LTILES, 128], dt.bfloat16)
            nc.sync.dma_start(lhsT_sb[:], lhsT_d[:])
            bias_sb = singles.tile([128, NSG * 2], dt.float32)
            nc.sync.dma_start(bias_sb[:], bias[:])

            def body(bi=0):
                for s in range(NSG):
                    for h in range(H):
                        g = gpool.tile([128, UTILES, HROWS], dt.float8e3)
                        for k, (cstart, clen) in enumerate(CALLS):
                            nc.gpsimd.dma_gather(
                                g[:, cstart // 128:(cstart + clen) // 128, :],
                                xt[:, h * HROWS:(h + 1) * HROWS],
                                idxs_sb[:, s * (UPAD // 16) + cstart // 16:
                                        s * (UPAD // 16) + (cstart + clen) // 16],
                                clen, clen, HROWS,
                                elem_step=N,
                                queue_num=k,
                            )
                        lbase = 0
                        for ot, (ustart, ucnt) in enumerate(WIN):
                            o = opool.tile([128, HROWS], dt.float16)
                            for ch in range(HROWS // 512):
                                p = ppool.tile([128, 512], dt.float32)
                                for u in range(ucnt):
                                    nc.tensor.matmul(
                                        p[:], lhsT_sb[:, s, lbase + u, :],
                                        g[:, ustart + u,
                                          ch * 512:(ch + 1) * 512],
                                        start=(u == 0), stop=(u == ucnt - 1))
                                nc.scalar.activation(
                                    o[:, ch * 512:(ch + 1) * 512], p[:],
                                    mybir.ActivationFunctionType.Identity,
                                    bias=bias_sb[:, s * 2 + ot:s * 2 + ot + 1])
                            nc.sync.dma_start(
                                outT[(s * 2 + ot) * 128:(s * 2 + ot + 1) * 128,
                                     h * HROWS:(h + 1) * HROWS],
                                o[:])
                            lbase += ucnt

            nloop, rem = divmod(reps, UNROLL)
            if nloop > 0:
                with tc.For_i(0, nloop, 1):
                    for bi in range(UNROLL):
                        body(bi)
            for bi in range(rem):
                body(bi)

    nc.compile()
    return nc


def _prep_inputs(x, in_index_per_out, weight, bias):
    """Host prep: fp8 xT (replicated) + per-core dedup tables."""
    idx = np.asarray(in_index_per_out).astype(np.int64)
    w = np.asarray(weight).astype(np.float32)
    b = np.asarray(bias).astype(np.float32)

    xT = np.ascontiguousarray(np.asarray(x).astype(np.float32).T
                              .astype(ml_dtypes.float8_e3m4))  # (IN_F, N)

    idxs_l, lhsT_l, bias_l = [], [], []
    for d in range(NCORES):
        idx_wraps = []
        lh_core = np.zeros((128, NSG, LTILES, 128), dtype=np.float32)
        for s in range(NSG):
            lo = d * OSLAB + s * SG
            iA, iB = idx[lo:lo + 128], idx[lo + 128:lo + 256]
            wA, wB = w[lo:lo + 128], w[lo + 128:lo + 256]
            uA, uB = np.unique(iA.ravel()), np.unique(iB.ravel())
            shared = np.intersect1d(uA, uB)
            Aonly = np.setdiff1d(uA, shared)
            Bonly = np.setdiff1d(uB, shared)
            ndup = max(0, len(shared) - MREG)
            excess, shmid = shared[:ndup], shared[ndup:]
            nAo, nBo, nSh = len(Aonly), len(Bonly), len(shmid)
            assert nAo + ndup <= AREG, (d, s, nAo, ndup)
            assert nBo + ndup <= UPAD - AREG - MREG, (d, s, nBo, ndup)
            # slot layout: [Aonly dupA pad | shmid pad | dupB Bonly pad]
            taps = np.zeros(UPAD, dtype=np.int64)
            taps[:nAo] = Aonly
            taps[nAo:nAo + ndup] = excess
            taps[AREG:AREG + nSh] = shmid
            taps[AREG + MREG:AREG + MREG + ndup] = excess
            taps[AREG + MREG + ndup:AREG + MREG + ndup + nBo] = Bonly
            # per-out-tile position maps (slot index for each feature)
            posA = np.full(IN_F, -1, dtype=np.int64)
            posA[Aonly] = np.arange(nAo)
            posA[excess] = nAo + np.arange(ndup)
            posA[shmid] = AREG + np.arange(nSh)
            posB = np.full(IN_F, -1, dtype=np.int64)
            posB[shmid] = AREG + np.arange(nSh)
            posB[excess] = AREG + MREG + np.arange(ndup)
            posB[Bonly] = AREG + MREG + ndup + np.arange(nBo)
            for ot, (ustart, ucnt), pm, io, wo in (
                    (0, WIN[0], posA, iA, wA), (1, WIN[1], posB, iB, wB)):
                Am = np.zeros((ucnt * 128, 128), dtype=np.float32)
                pos = pm[io] - ustart * 128       # (128, 16)
                assert (pos >= 0).all() and (pos < ucnt * 128).all(), (d, s, ot)
                for ol in range(128):
                    for m in range(K):
                        Am[pos[ol, m], ol] += wo[ol, m]
                lbase = 0 if ot == 0 else WIN[0][1]
                lh_core[:, s, lbase:lbase + ucnt, :] = \
                    Am.reshape(ucnt, 128, 128).transpose(1, 0, 2)
            idx_wraps.append(np.tile(
                taps.astype(np.int16).reshape(-1, 16).T, (8, 1)))
        idxs_l.append(np.ascontiguousarray(
            np.concatenate(idx_wraps, axis=1)).astype(np.int16))
        lhsT_l.append(np.ascontiguousarray(
            lh_core.reshape(128, NSG * LTILES * 128))
            .astype(ml_dtypes.bfloat16))
        bias_l.append(np.ascontiguousarray(
            b[d * OSLAB:(d + 1) * OSLAB].reshape(NSG * 2, 128).T))
    return xT, idxs_l, lhsT_l, bias_l


def _in_maps(inputs):
    xT, idxs_l, lhsT_l, bias_l = _prep_inputs(
        inputs["x"], inputs["in_index_per_out"], inputs["weight"],
        inputs["bias"])
    return [
        {"xt": xT, "idxs": idxs_l[d], "lhsT": lhsT_l[d], "bias": bias_l[d]}
        for d in range(NCORES)
    ]


def kernel(x, in_index_per_out, weight, bias):
    from concourse import bass_utils

    in_maps = _in_maps({"x": x, "in_index_per_out": in_index_per_out,
                        "weight": weight, "bias": bias})

    if "nc" not in _CACHE:
        _CACHE["nc"] = _build(reps=1)
    nc = _CACHE["nc"]
    res = bass_utils.run_bass_kernel_spmd(nc, in_maps,
                                          core_ids=list(range(NCORES)))
    out = np.empty((N, OUT_F), dtype=np.float32)
    for d in range(NCORES):
        out[:, d * OSLAB:(d + 1) * OSLAB] = \
            res.results[d]["outT"].astype(np.float32).T
    return out
